# revision 23
# baseline (speedup 1.0000x reference)
"""Trainium2 Bass kernel for nn_LocalExperts (MoE grouped FFN).

out[e] = relu(x[e] @ wi[e]) @ wo[e]   for e in 0..7

Expert-parallel over 8 NeuronCores: core e computes expert e's FFN.
Per-core work: x [8192, 512] f32, wi [512, 2048], wo [2048, 512]
  GEMM1: hT[f, m] = wi[d, f].T @ xT[d, m]  (accumulate over 4 d-chunks)
  relu (ScalarE) -> hT in SBUF
  GEMM2: out[m, d] = hT[f, m].T @ wo[f, d] (accumulate over 16 f-chunks)

All GEMM operands are bf16 (PSUM accumulation stays fp32): same PE
streaming rate as fp32r but half the SBUF/xbus operand traffic, FWL
weight loads, and lower power (less P0 downclock risk).  Max-abs error
stays ~25x under the 2e-2 gate.

Structure vs the 537us fp32r baseline:
  - Steady-state x transposes run on the DVE: ScalarE casts x to bf16,
    then 16 cross-quadrant 32x32-block STREAM_TRANSPOSEs (2-byte = 2x
    DVE rate).  The PE stream is pure GEMM matmuls.
  - Startup: x0 DMA'd first (per-ms chunks), a short PE warmup holds
    the HAM clock gate at K=8/8, and tile 0 is transposed on the
    then-idle PE.
  - Weights DMA as fp32 into staging and are cast to bf16 on the DVE
    (chunked, overlapped with early compute).
  - GEMM2 drains 2 PSUM banks per DVE copy / output DMA; the very last
    drain splits across DVE+ScalarE with two parallel DMAs to shorten
    the tail.
"""

import numpy as np

import concourse.mybir as mybir
from concourse import bacc
from concourse.tile import TileContext
from concourse.bass_utils import run_bass_kernel_spmd
from concourse.masks import make_identity

E, W, C, D, F = 8, 8, 1024, 512, 2048
P = 128
M_TOT = W * C            # 8192 rows per expert
M_TILE = 512             # rows per m-tile
N_MT = M_TOT // M_TILE   # 16
MS = M_TILE // P         # 4 m-subtiles of 128 rows
DC = D // P              # 4 d-chunks
FC = F // P              # 16 f-chunks
Q = 32                   # DVE stream-transpose square size

F32 = mybir.dt.float32
F32R = mybir.dt.float32r
BF16 = mybir.dt.bfloat16
WARM_MMS = 20            # PE warmup matmuls: bridge until x0's DMA lands


def _build_nc():
    nc = bacc.Bacc(None, target_bir_lowering=False)

    x = nc.dram_tensor("x", [M_TOT, D], F32, kind="ExternalInput")
    wi = nc.dram_tensor("wi", [D, F], F32, kind="ExternalInput")
    wo = nc.dram_tensor("wo", [F, D], F32, kind="ExternalInput")
    out = nc.dram_tensor("out", [M_TOT, D], F32, kind="ExternalOutput")

    x_v = x.rearrange("(mt ms p) d -> mt p ms d", p=P, ms=MS)
    out_v = out.rearrange("(mt ms p) d -> mt p ms d", p=P, ms=MS)
    wi_v = wi.rearrange("(dc p) f -> p dc f", p=P)
    wo_v = wo.rearrange("(fc p) d -> p fc d", p=P)

    with TileContext(nc) as tc:
        with (
            tc.tile_pool(name="const", bufs=1) as cpool,
            tc.tile_pool(name="xin", bufs=2) as xin_pool,
            tc.tile_pool(name="xb", bufs=2) as xb_pool,
            tc.tile_pool(name="xt", bufs=2) as xt_pool,
            tc.tile_pool(name="ht", bufs=2) as ht_pool,
            tc.tile_pool(name="wst", bufs=2) as wst_pool,
            tc.tile_pool(name="osb", bufs=2) as o_pool,
            tc.tile_pool(name="h_ps", bufs=2, space="PSUM") as h_psum,
            tc.tile_pool(name="o_ps", bufs=2, space="PSUM") as o_psum,
        ):
            # PE warmup: dummy bf16 matmuls on memset tiles keep the HAM
            # clock gate at K=8/8 while x0's DMA is in flight.  The PSUM
            # bank tag-shares with the GEMM2 drain pool (used much later).
            wlhs = cpool.tile([P, P], BF16)
            wrhs = cpool.tile([P, M_TILE], BF16)
            nc.gpsimd.memset(wlhs, 0)
            nc.gpsimd.memset(wrhs, 0)
            wps = o_psum.tile([P, 2, D], F32, tag="o")
            for _ in range(WARM_MMS):
                nc.tensor.matmul(wps[:, 0], wlhs, wrhs, start=True, stop=True)

            ident = cpool.tile([P, P], F32)
            make_identity(nc, ident)

            # x tile 0 first (per-ms chunks): it gates tile-0 transposes.
            x_nat0 = xin_pool.tile([P, MS, D], F32)
            for ms in range(MS):
                nc.sync.dma_start(x_nat0[:, ms], x_v[0][:, ms])

            # Weights: fp32 DMA into staging (tag-shared), DVE cast to
            # bf16, chunked so GEMM1's first fc-groups unblock early.
            wi_sb = cpool.tile([P, DC, F], BF16)
            wo_sb = cpool.tile([P, FC, D], BF16)
            FQ = 4
            # wi casts run on ScalarE (idle until the first relu); wo's
            # DMA+cast issue inside the first loop iterations so their DVE
            # priority is below the tile-1/2 transposes.
            wi_st0 = wst_pool.tile([P, DC, F // FQ], F32, tag="w")
            nc.sync.dma_start(wi_st0, wi_v[:, :, : F // FQ])
            nc.scalar.activation(
                wi_sb[:, :, : F // FQ], wi_st0,
                mybir.ActivationFunctionType.Copy,
            )

            def load_x(mt):
                x_nat = xin_pool.tile([P, MS, D], F32)
                nc.sync.dma_start(x_nat, x_v[mt])
                return x_nat

            x_next = load_x(1)
            for q in range(1, FQ):
                fs = slice(q * (F // FQ), (q + 1) * (F // FQ))
                wi_st = wst_pool.tile([P, DC, F // FQ], F32, tag="w")
                nc.sync.dma_start(wi_st, wi_v[:, :, fs])
                nc.scalar.activation(
                    wi_sb[:, :, fs], wi_st,
                    mybir.ActivationFunctionType.Copy,
                )

            def load_wo(q):
                s = slice(q * (FC // 4), (q + 1) * (FC // 4))
                wo_st = wst_pool.tile([P, FC // 4, D], F32, tag="w")
                nc.sync.dma_start(wo_st, wo_v[:, s])
                nc.vector.tensor_copy(wo_sb[:, s], wo_st)

            def transpose_x0_pe(x_nat):
                # Tile 0 only: PE-mode transpose (the PE is idle before the
                # first GEMM).  4 transposes per ms form one PSUM group in a
                # bank of an h-tagged 2-bank tile, drained by one DVE CAST
                # (fp32 PSUM -> bf16 xt).
                xt = xt_pool.tile([P, DC, M_TILE], BF16)
                tp = h_psum.tile([P, 2, M_TILE], F32, tag="h")
                tpv = tp.rearrange("p b (dc q) -> p b dc q", q=P)
                for ms in range(MS):
                    b = ms % 2
                    for dc in range(DC):
                        nc.tensor.matmul(
                            tpv[:, b, dc],
                            x_nat[:, ms, dc * P : (dc + 1) * P],
                            ident,
                            is_transpose=True,
                            start=(dc == 0),
                            stop=(dc == DC - 1),
                            skip_group_check=True,
                        )
                    nc.vector.tensor_copy(
                        xt[:, :, ms * P : (ms + 1) * P], tpv[:, b]
                    )
                return xt

            def transpose_x(x_nat):
                # Steady state: ScalarE casts x fp32 -> bf16, then the DVE
                # transposes with 16 cross-quadrant STREAM_TRANSPOSEs, each
                # [32 parts, (ms, dc) x 32], at 2-byte (2x) rate.
                xb = xb_pool.tile([P, MS, D], BF16)
                nc.scalar.activation(
                    xb, x_nat, mybir.ActivationFunctionType.Copy
                )
                xt = xt_pool.tile([P, DC, M_TILE], BF16)
                xn = xb.rearrange("p ms (dc q) -> p ms dc q", q=P)
                xr = xt.rearrange("p dc (ms q) -> p ms dc q", q=P)
                for i in range(P // Q):      # output d-group (partitions)
                    for j in range(P // Q):  # input m-group (partitions)
                        nc.vector.transpose(
                            xr[Q * i : Q * (i + 1), :, :, Q * j : Q * (j + 1)],
                            xn[Q * j : Q * (j + 1), :, :, Q * i : Q * (i + 1)],
                        )
                return xt

            def gemm1(xt):
                # hT[f, m]; two 4-matmul PSUM groups (adjacent banks of one
                # 2-bank tile) drained by a single ACT relu -> bf16 SBUF.
                hT = ht_pool.tile([P, FC, M_TILE], BF16)
                for fc2 in range(FC // 2):
                    hp = h_psum.tile([P, 2, M_TILE], F32, tag="h")
                    for half in range(2):
                        fc = 2 * fc2 + half
                        for dc in range(DC):
                            nc.tensor.matmul(
                                hp[:, half],
                                wi_sb[:, dc, fc * P : (fc + 1) * P],
                                xt[:, dc, :],
                                start=(dc == 0),
                                stop=(dc == DC - 1),
                            )
                    nc.scalar.activation(
                        hT[:, 2 * fc2 : 2 * fc2 + 2, :],
                        hp,
                        mybir.ActivationFunctionType.Relu,
                    )
                return hT

            def gemm2(mt, hT):
                # out[m, d]: two 128-row subtiles per PSUM tile (2 banks),
                # one DVE drain + one 512KB output DMA per pair.
                final = mt == N_MT - 1
                for mh in range(MS // 2):
                    op = o_psum.tile([P, 2, D], F32, tag="o")
                    o_t = o_pool.tile([P, 2, D], F32)
                    for s in range(2):
                        ms = 2 * mh + s
                        for fc in range(FC):
                            nc.tensor.matmul(
                                op[:, s],
                                hT[:, fc, ms * P : (ms + 1) * P],
                                wo_sb[:, fc, :],
                                start=(fc == 0),
                                stop=(fc == FC - 1),
                            )
                    if final and mh == MS // 2 - 1:
                        # very last drain: DVE and ScalarE copy one bank
                        # each concurrently, two parallel 256KB DMAs.
                        nc.vector.tensor_copy(o_t[:, 0], op[:, 0])
                        nc.scalar.activation(
                            o_t[:, 1], op[:, 1],
                            mybir.ActivationFunctionType.Copy,
                        )
                        nc.sync.dma_start(out_v[mt, :, 2 * mh, :], o_t[:, 0])
                        nc.sync.dma_start(out_v[mt, :, 2 * mh + 1, :], o_t[:, 1])
                    else:
                        nc.vector.tensor_copy(o_t, op)
                        nc.sync.dma_start(out_v[mt, :, 2 * mh : 2 * mh + 2, :], o_t)

            # software pipeline: DVE transposes m-tile t+1 while the PE runs
            # GEMM1(t)/GEMM2(t); x DMA prefetches t+2.
            xt = transpose_x0_pe(x_nat0)
            for mt in range(N_MT):
                hT = gemm1(xt)
                if mt + 1 < N_MT:
                    xt = transpose_x(x_next)
                    if mt == 0:
                        for q in range(4):
                            load_wo(q)
                    if mt + 2 < N_MT:
                        x_next = load_x(mt + 2)
                gemm2(mt, hT)

    nc.finalize()
    return nc


_CACHE = {}


def _get_nc():
    if "nc" not in _CACHE:
        _CACHE["nc"] = _build_nc()
    return _CACHE["nc"]


def _run(x, wi, wo, **spmd_kwargs):
    """x [E, 8192, 512], wi [E, 512, 2048], wo [E, 2048, 512] -> results."""
    nc = _get_nc()
    in_maps = [
        {
            "x": np.ascontiguousarray(x[e]),
            "wi": np.ascontiguousarray(wi[e]),
            "wo": np.ascontiguousarray(wo[e]),
        }
        for e in range(E)
    ]
    return nc, run_bass_kernel_spmd(nc, in_maps, core_ids=list(range(E)), **spmd_kwargs)


def kernel(dispatched_hidden_states, experts_capacity_usage=None, wi=None, wo=None):
    x = np.asarray(dispatched_hidden_states, dtype=np.float32).reshape(E, M_TOT, D)
    wi_ = np.asarray(wi, dtype=np.float32)
    wo_ = np.asarray(wo, dtype=np.float32)
    _, res = _run(x, wi_, wo_)
    out = np.stack([res.results[e]["out"] for e in range(E)])
    return out.reshape(E, W, C, D)


# revision 24
# speedup vs baseline: 1.0004x; 1.0004x over previous
"""Trainium2 Bass kernel for nn_LocalExperts (MoE grouped FFN).

out[e] = relu(x[e] @ wi[e]) @ wo[e]   for e in 0..7

Expert-parallel over 8 NeuronCores: core e computes expert e's FFN.
Per-core work: x [8192, 512] f32, wi [512, 2048], wo [2048, 512]
  GEMM1: hT[f, m] = wi[d, f].T @ xT[d, m]  (accumulate over 4 d-chunks)
  relu (ScalarE) -> hT in SBUF
  GEMM2: out[m, d] = hT[f, m].T @ wo[f, d] (accumulate over 16 f-chunks)

All GEMM operands are bf16 (PSUM accumulation stays fp32): same PE
streaming rate as fp32r but half the SBUF/xbus operand traffic, FWL
weight loads, and lower power (less P0 downclock risk).  Max-abs error
stays ~25x under the 2e-2 gate.

Structure vs the 537us fp32r baseline:
  - Steady-state x transposes run on the DVE: ScalarE casts x to bf16,
    then 16 cross-quadrant 32x32-block STREAM_TRANSPOSEs (2-byte = 2x
    DVE rate).  The PE stream is pure GEMM matmuls.
  - Startup: x0 DMA'd first (per-ms chunks), a short PE warmup holds
    the HAM clock gate at K=8/8, and tile 0 is transposed on the
    then-idle PE.
  - Weights DMA as fp32 into staging and are cast to bf16 on the DVE
    (chunked, overlapped with early compute).
  - GEMM2 drains 2 PSUM banks per DVE copy / output DMA; the very last
    drain splits across DVE+ScalarE with two parallel DMAs to shorten
    the tail.
"""

import numpy as np

import concourse.mybir as mybir
from concourse import bacc
from concourse.tile import TileContext
from concourse.bass_utils import run_bass_kernel_spmd
from concourse.masks import make_identity

E, W, C, D, F = 8, 8, 1024, 512, 2048
P = 128
M_TOT = W * C            # 8192 rows per expert
M_TILE = 512             # rows per m-tile
N_MT = M_TOT // M_TILE   # 16
MS = M_TILE // P         # 4 m-subtiles of 128 rows
DC = D // P              # 4 d-chunks
FC = F // P              # 16 f-chunks
Q = 32                   # DVE stream-transpose square size

F32 = mybir.dt.float32
F32R = mybir.dt.float32r
BF16 = mybir.dt.bfloat16
WARM_MMS = 48            # PE warmup: bridge x0's DMA + tile-0 transposes (keeps HAM warm)


def _build_nc():
    nc = bacc.Bacc(None, target_bir_lowering=False)

    x = nc.dram_tensor("x", [M_TOT, D], F32, kind="ExternalInput")
    wi = nc.dram_tensor("wi", [D, F], F32, kind="ExternalInput")
    wo = nc.dram_tensor("wo", [F, D], F32, kind="ExternalInput")
    out = nc.dram_tensor("out", [M_TOT, D], F32, kind="ExternalOutput")

    x_v = x.rearrange("(mt ms p) d -> mt p ms d", p=P, ms=MS)
    out_v = out.rearrange("(mt ms p) d -> mt p ms d", p=P, ms=MS)
    wi_v = wi.rearrange("(dc p) f -> p dc f", p=P)
    wo_v = wo.rearrange("(fc p) d -> p fc d", p=P)

    with TileContext(nc) as tc:
        with (
            tc.tile_pool(name="const", bufs=1) as cpool,
            tc.tile_pool(name="xin", bufs=2) as xin_pool,
            tc.tile_pool(name="xb", bufs=2) as xb_pool,
            tc.tile_pool(name="xt", bufs=2) as xt_pool,
            tc.tile_pool(name="ht", bufs=2) as ht_pool,
            tc.tile_pool(name="wst", bufs=2) as wst_pool,
            tc.tile_pool(name="osb", bufs=2) as o_pool,
            tc.tile_pool(name="h_ps", bufs=2, space="PSUM") as h_psum,
            tc.tile_pool(name="o_ps", bufs=2, space="PSUM") as o_psum,
        ):
            # PE warmup: dummy bf16 matmuls on memset tiles keep the HAM
            # clock gate at K=8/8 while x0's DMA is in flight.  The PSUM
            # bank tag-shares with the GEMM2 drain pool (used much later).
            wlhs = cpool.tile([P, P], BF16)
            wrhs = cpool.tile([P, M_TILE], BF16)
            nc.gpsimd.memset(wlhs, 0)
            nc.gpsimd.memset(wrhs, 0)
            wps = o_psum.tile([P, 2, D], F32, tag="o")
            for _ in range(WARM_MMS):
                nc.tensor.matmul(wps[:, 0], wlhs, wrhs, start=True, stop=True)

            ident = cpool.tile([P, P], F32)
            make_identity(nc, ident)

            # x tile 0 first (per-ms chunks): it gates tile-0 transposes.
            x_nat0 = xin_pool.tile([P, MS, D], F32)
            for ms in range(MS):
                nc.sync.dma_start(x_nat0[:, ms], x_v[0][:, ms])

            # Weights: fp32 DMA into staging (tag-shared), DVE cast to
            # bf16, chunked so GEMM1's first fc-groups unblock early.
            wi_sb = cpool.tile([P, DC, F], BF16)
            wo_sb = cpool.tile([P, FC, D], BF16)
            FQ = 4
            # wi casts run on ScalarE (idle until the first relu); wo's
            # DMA+cast issue inside the first loop iterations so their DVE
            # priority is below the tile-1/2 transposes.
            wi_st0 = wst_pool.tile([P, DC, F // FQ], F32, tag="w")
            nc.sync.dma_start(wi_st0, wi_v[:, :, : F // FQ])
            nc.scalar.activation(
                wi_sb[:, :, : F // FQ], wi_st0,
                mybir.ActivationFunctionType.Copy,
            )

            def load_x(mt):
                x_nat = xin_pool.tile([P, MS, D], F32)
                nc.sync.dma_start(x_nat, x_v[mt])
                return x_nat

            x_next = load_x(1)
            for q in range(1, FQ):
                fs = slice(q * (F // FQ), (q + 1) * (F // FQ))
                wi_st = wst_pool.tile([P, DC, F // FQ], F32, tag="w")
                nc.sync.dma_start(wi_st, wi_v[:, :, fs])
                nc.scalar.activation(
                    wi_sb[:, :, fs], wi_st,
                    mybir.ActivationFunctionType.Copy,
                )

            def load_wo(q):
                s = slice(q * (FC // 4), (q + 1) * (FC // 4))
                wo_st = wst_pool.tile([P, FC // 4, D], F32, tag="w")
                nc.sync.dma_start(wo_st, wo_v[:, s])
                if q < 2:
                    nc.vector.tensor_copy(wo_sb[:, s], wo_st)
                else:
                    nc.scalar.activation(
                        wo_sb[:, s], wo_st,
                        mybir.ActivationFunctionType.Copy,
                    )

            for q in range(4):
                load_wo(q)

            def transpose_x0_pe(x_nat):
                # Tile 0 only: PE-mode transpose (the PE is idle before the
                # first GEMM).  4 transposes per ms form one PSUM group in a
                # bank of an h-tagged 2-bank tile, drained by one DVE CAST
                # (fp32 PSUM -> bf16 xt).
                xt = xt_pool.tile([P, DC, M_TILE], BF16)
                tp = h_psum.tile([P, 2, M_TILE], F32, tag="h")
                tpv = tp.rearrange("p b (dc q) -> p b dc q", q=P)
                for ms in range(MS):
                    b = ms % 2
                    for dc in range(DC):
                        nc.tensor.matmul(
                            tpv[:, b, dc],
                            x_nat[:, ms, dc * P : (dc + 1) * P],
                            ident,
                            is_transpose=True,
                            start=(dc == 0),
                            stop=(dc == DC - 1),
                            skip_group_check=True,
                        )
                    nc.vector.tensor_copy(
                        xt[:, :, ms * P : (ms + 1) * P], tpv[:, b]
                    )
                return xt

            def transpose_x(x_nat):
                # Steady state: ScalarE casts x fp32 -> bf16, then the DVE
                # transposes with 16 cross-quadrant STREAM_TRANSPOSEs, each
                # [32 parts, (ms, dc) x 32], at 2-byte (2x) rate.
                xb = xb_pool.tile([P, MS, D], BF16)
                nc.scalar.activation(
                    xb, x_nat, mybir.ActivationFunctionType.Copy
                )
                xt = xt_pool.tile([P, DC, M_TILE], BF16)
                xn = xb.rearrange("p ms (dc q) -> p ms dc q", q=P)
                xr = xt.rearrange("p dc (ms q) -> p ms dc q", q=P)
                for i in range(P // Q):      # output d-group (partitions)
                    for j in range(P // Q):  # input m-group (partitions)
                        nc.vector.transpose(
                            xr[Q * i : Q * (i + 1), :, :, Q * j : Q * (j + 1)],
                            xn[Q * j : Q * (j + 1), :, :, Q * i : Q * (i + 1)],
                        )
                return xt

            def gemm1(xt):
                # hT[f, m]; two 4-matmul PSUM groups (adjacent banks of one
                # 2-bank tile) drained by a single ACT relu -> bf16 SBUF.
                hT = ht_pool.tile([P, FC, M_TILE], BF16)
                for fc2 in range(FC // 2):
                    hp = h_psum.tile([P, 2, M_TILE], F32, tag="h")
                    for half in range(2):
                        fc = 2 * fc2 + half
                        for dc in range(DC):
                            nc.tensor.matmul(
                                hp[:, half],
                                wi_sb[:, dc, fc * P : (fc + 1) * P],
                                xt[:, dc, :],
                                start=(dc == 0),
                                stop=(dc == DC - 1),
                            )
                    nc.scalar.activation(
                        hT[:, 2 * fc2 : 2 * fc2 + 2, :],
                        hp,
                        mybir.ActivationFunctionType.Relu,
                    )
                return hT

            def gemm2(mt, hT):
                # out[m, d]: two 128-row subtiles per PSUM tile (2 banks),
                # one DVE drain + one 512KB output DMA per pair.
                final = mt == N_MT - 1
                for mh in range(MS // 2):
                    op = o_psum.tile([P, 2, D], F32, tag="o")
                    o_t = o_pool.tile([P, 2, D], F32)
                    for s in range(2):
                        ms = 2 * mh + s
                        for fc in range(FC):
                            nc.tensor.matmul(
                                op[:, s],
                                hT[:, fc, ms * P : (ms + 1) * P],
                                wo_sb[:, fc, :],
                                start=(fc == 0),
                                stop=(fc == FC - 1),
                            )
                    if final and mh == MS // 2 - 1:
                        # very last drain: DVE and ScalarE copy one bank
                        # each concurrently, two parallel 256KB DMAs.
                        nc.vector.tensor_copy(o_t[:, 0], op[:, 0])
                        nc.scalar.activation(
                            o_t[:, 1], op[:, 1],
                            mybir.ActivationFunctionType.Copy,
                        )
                        nc.sync.dma_start(out_v[mt, :, 2 * mh, :], o_t[:, 0])
                        nc.sync.dma_start(out_v[mt, :, 2 * mh + 1, :], o_t[:, 1])
                    else:
                        nc.vector.tensor_copy(o_t, op)
                        nc.sync.dma_start(out_v[mt, :, 2 * mh : 2 * mh + 2, :], o_t)

            # software pipeline: DVE transposes m-tile t+1 while the PE runs
            # GEMM1(t)/GEMM2(t); x DMA prefetches t+2.
            xt = transpose_x0_pe(x_nat0)
            for mt in range(N_MT):
                hT = gemm1(xt)
                if mt + 1 < N_MT:
                    xt = transpose_x(x_next)
                    if mt + 2 < N_MT:
                        x_next = load_x(mt + 2)
                gemm2(mt, hT)

    nc.finalize()
    return nc


_CACHE = {}


def _get_nc():
    if "nc" not in _CACHE:
        _CACHE["nc"] = _build_nc()
    return _CACHE["nc"]


def _run(x, wi, wo, **spmd_kwargs):
    """x [E, 8192, 512], wi [E, 512, 2048], wo [E, 2048, 512] -> results."""
    nc = _get_nc()
    in_maps = [
        {
            "x": np.ascontiguousarray(x[e]),
            "wi": np.ascontiguousarray(wi[e]),
            "wo": np.ascontiguousarray(wo[e]),
        }
        for e in range(E)
    ]
    return nc, run_bass_kernel_spmd(nc, in_maps, core_ids=list(range(E)), **spmd_kwargs)


def kernel(dispatched_hidden_states, experts_capacity_usage=None, wi=None, wo=None):
    x = np.asarray(dispatched_hidden_states, dtype=np.float32).reshape(E, M_TOT, D)
    wi_ = np.asarray(wi, dtype=np.float32)
    wo_ = np.asarray(wo, dtype=np.float32)
    _, res = _run(x, wi_, wo_)
    out = np.stack([res.results[e]["out"] for e in range(E)])
    return out.reshape(E, W, C, D)
